# revision 1
# baseline (speedup 1.0000x reference)
"""Trainium2 Bass kernel for nn_CovBlock (B=4, N=8192, D=2048, H=512, F=64).

Data-parallel over 8 NeuronCores: x sharded along N (1024 rows/batch/core).
Per core: one streaming pass over its x shard computing per-column
sum-of-squares of row-centered x, accumulated per batch in PSUM via
TensorE matmuls with a one-hot stationary operand.  Partial ss is
PE-transposed, AllGathered and summed locally; cov = ss/(ss+eps) feeds a
3-layer MLP that is column-sharded (W1) / row-sharded (W2) across cores
with a second AllGather for the layer-2 partial sums.
"""

import sys

sys.path.insert(0, "/opt/trn_rl_repo")

import numpy as np

B, N, D, H, F = 4, 8192, 2048, 512, 64
NCORES = 8
P = 128
EPS = 1e-6
SLOPE = 0.01

_CACHE = {}


def _build(nsh, debug=False, chunk_tiles=4, xbufs=3, sqbufs=3):
    import concourse.bacc as bacc
    import concourse.mybir as mybir
    from concourse import tile

    dt = mybir.dt.float32
    bt = mybir.dt.bfloat16
    AF = mybir.ActivationFunctionType
    ROWS = B * nsh
    NT = ROWS // P            # total 128-row tiles per core
    TPB = nsh // P            # tiles per batch
    KC = D // P               # 16 k-chunks of 128
    JSL = D // NCORES         # 256: L1 output column slice per core
    J2C = JSL // P            # 2:  L1-slice k-chunks for L2
    HC = H // P               # 4:  H chunks of 128
    CT = min(chunk_tiles, NT)
    NCH = NT // CT
    assert NT % CT == 0 and nsh % P == 0

    nc = bacc.Bacc("TRN2", target_bir_lowering=False, debug=False,
                   num_devices=NCORES)

    x = nc.dram_tensor("x", [ROWS, D], dt, kind="ExternalInput")
    w1t = nc.dram_tensor("w1t", [P, KC, JSL], bt, kind="ExternalInput")
    w2t = nc.dram_tensor("w2t", [P, J2C, H], bt, kind="ExternalInput")
    w3t = nc.dram_tensor("w3t", [P, HC, F], bt, kind="ExternalInput")
    b1r = nc.dram_tensor("b1r", [1, JSL], bt, kind="ExternalInput")
    b2tin = nc.dram_tensor("b2tin", [P, HC], dt, kind="ExternalInput")
    b3r = nc.dram_tensor("b3r", [1, F], bt, kind="ExternalInput")
    ident = nc.dram_tensor("ident", [B, B], dt, kind="ExternalInput")
    identb = nc.dram_tensor("identb", [B, B], bt, kind="ExternalInput")
    out = nc.dram_tensor("out", [B, F], dt, kind="ExternalOutput")
    dbg = {}
    if debug:
        for name, shape in [("dbg_ssp", [P, KC * B]), ("dbg_ssum", [P, KC * B])]:
            dbg[name] = nc.dram_tensor(name, shape, dt, kind="ExternalOutput")

    groups = [list(range(NCORES))]

    with tile.TileContext(nc) as tc:
        with (
            tc.tile_pool(name="xp", bufs=xbufs) as xp,
            tc.tile_pool(name="sq", bufs=sqbufs) as sq,
            tc.tile_pool(name="sm", bufs=6) as sm,
            tc.tile_pool(name="wp", bufs=1) as wp,
            tc.tile_pool(name="tl", bufs=1) as tl,
            tc.tile_pool(name="pp", bufs=1, space="PSUM") as pp,
            tc.tile_pool(name="dr", bufs=1, space="DRAM") as dr,
        ):
            # constants
            onehots = wp.tile([P, B * B], bt)
            nc.any.memset(onehots[:], 0.0)
            for b in range(B):
                nc.any.memset(onehots[:, b * B + b:b * B + b + 1], 1.0)
            ident4 = wp.tile([B, B], dt)
            nc.gpsimd.dma_start(ident4[:], ident.ap()[:, :])
            ident4b = wp.tile([B, B], bt)
            nc.gpsimd.dma_start(ident4b[:], identb.ap()[:, :])
            ones14 = wp.tile([1, B], bt)
            nc.any.memset(ones14[:], 1.0)

            ss_psum = pp.tile([B, D], dt)

            # weight/bias prefetch on the ACT HWDGE ring (SP ring carries x)
            w1sb = wp.tile([P, KC, JSL], bt)
            w2sb = wp.tile([P, J2C, H], bt)
            w3sb = wp.tile([P, HC, F], bt)
            b1row = wp.tile([1, JSL], bt)
            b2T = wp.tile([P, HC], dt)
            b3row = wp.tile([1, F], bt)
            nc.gpsimd.dma_start(w1sb[:], w1t.ap()[:, :, :])
            nc.gpsimd.dma_start(w2sb[:], w2t.ap()[:, :, :])
            nc.gpsimd.dma_start(w3sb[:], w3t.ap()[:, :, :])
            nc.gpsimd.dma_start(b1row[:], b1r.ap()[:, :])
            nc.gpsimd.dma_start(b2T[:], b2tin.ap()[:, :])
            nc.gpsimd.dma_start(b3row[:], b3r.ap()[:, :])

            # ---- main pass over x ----
            for k in range(NCH):
                xch = xp.tile([P, CT, D], dt)
                src = x.ap()[k * CT * P:(k + 1) * CT * P, :]
                nc.sync.dma_start(xch[:], src.rearrange("(t p) d -> p t d", p=P))
                for t in range(CT):
                    g = k * CT + t
                    b, tib = g // TPB, g % TPB
                    xt = xch[:, t, :]
                    negsum = sm.tile([P, 1], dt)
                    nc.vector.reduce_sum(negsum[:], xt, axis=mybir.AxisListType.X)
                    negmu = sm.tile([P, 1], dt)
                    nc.gpsimd.tensor_scalar_mul(negmu[:], negsum[:], -1.0 / D)
                    xsq = sq.tile([P, D], bt)
                    nc.scalar.activation(xsq[:], xt, AF.Square,
                                         bias=negmu[:], scale=1.0)
                    for q in range(D // 512):
                        nc.tensor.matmul(
                            ss_psum[:, q * 512:(q + 1) * 512],
                            lhsT=onehots[:, b * B:(b + 1) * B],
                            rhs=xsq[:, q * 512:(q + 1) * 512],
                            start=(g == 0), stop=(g == NT - 1))

            # ---- tail: ss -> cov (transposed layout) ----
            ss_sb = tl.tile([B, D], dt)
            nc.vector.tensor_copy(ss_sb[:, :D // 2], ss_psum[:, :D // 2])
            nc.scalar.copy(ss_sb[:, D // 2:], ss_psum[:, D // 2:])
            ssT_psum = pp.tile([P, KC * B], dt, tag="tps", bufs=2)
            for c in range(KC):
                nc.tensor.transpose(ssT_psum[:, c * B:(c + 1) * B],
                                    ss_sb[0:B, c * P:(c + 1) * P], ident4[:])
            ssTp = tl.tile([P, KC * B], dt)
            nc.vector.tensor_copy(ssTp[:], ssT_psum[:])

            ss_in = dr.tile([P, KC * B], dt)
            ss_g = dr.tile([NCORES * P, KC * B], dt)
            nc.sync.dma_start(ss_in[:], ssTp[:])
            nc.gpsimd.collective_compute(
                "AllGather", mybir.AluOpType.bypass, replica_groups=groups,
                ins=[ss_in.opt()], outs=[ss_g.opt()])
            gsb = tl.tile([P, NCORES, KC * B], dt)
            nc.gpsimd.dma_start(gsb[:], ss_g.opt().rearrange("(i p) c -> p i c", p=P))
            ssum = tl.tile([P, KC * B], dt)
            nc.vector.reduce_sum(ssum[:], gsb[:].rearrange("p i c -> p c i"),
                                 axis=mybir.AxisListType.X)
            t1 = tl.tile([P, KC * B], dt)
            nc.vector.tensor_scalar_add(t1[:], ssum[:], EPS)
            t2 = tl.tile([P, KC * B], dt)
            nc.vector.reciprocal(t2[:], t1[:])
            cov = tl.tile([P, KC * B], bt)
            nc.vector.tensor_mul(cov[:], ssum[:], t2[:])

            # ---- L1: h1 = leaky(cov @ W1[:, slice] + b1[slice])  [B, JSL] ----
            h1_psum = pp.tile([B, JSL], dt, tag="tps", bufs=2)
            for c in range(KC):
                nc.tensor.matmul(h1_psum[:], lhsT=cov[:, c * B:(c + 1) * B],
                                 rhs=w1sb[:, c, :], start=(c == 0), stop=False)
            nc.tensor.matmul(h1_psum[:], lhsT=ones14[:], rhs=b1row[:],
                             start=False, stop=True)
            h1a = tl.tile([B, JSL], dt)
            nc.vector.tensor_scalar_mul(h1a[:], h1_psum[:], SLOPE)
            h1_sb = tl.tile([B, JSL], bt)
            nc.vector.tensor_max(h1_sb[:], h1_psum[:], h1a[:])

            h1T_psum = pp.tile([P, J2C * B], bt, tag="tps", bufs=2)
            for cc in range(J2C):
                nc.tensor.transpose(h1T_psum[:, cc * B:(cc + 1) * B],
                                    h1_sb[0:B, cc * P:(cc + 1) * P], ident4b[:])
            h1T = tl.tile([P, J2C * B], bt)
            nc.vector.tensor_copy(h1T[:], h1T_psum[:])

            # ---- L2 partial: h2p = h1 @ W2[slice, :]  [B, H] ----
            h2_psum = pp.tile([B, H], dt, tag="tps", bufs=2)
            for cc in range(J2C):
                nc.tensor.matmul(h2_psum[:], lhsT=h1T[:, cc * B:(cc + 1) * B],
                                 rhs=w2sb[:, cc, :], start=(cc == 0),
                                 stop=(cc == J2C - 1))
            h2p_sb = tl.tile([B, H], dt)
            nc.vector.tensor_copy(h2p_sb[:, :H // 2], h2_psum[:, :H // 2])
            nc.scalar.copy(h2p_sb[:, H // 2:], h2_psum[:, H // 2:])
            h2T_psum = pp.tile([P, HC * B], dt, tag="tps", bufs=2)
            for r in range(HC):
                nc.tensor.transpose(h2T_psum[:, r * B:(r + 1) * B],
                                    h2p_sb[0:B, r * P:(r + 1) * P], ident4[:])
            h2Tp = tl.tile([P, HC * B], dt)
            nc.vector.tensor_copy(h2Tp[:], h2T_psum[:])

            h2_in = dr.tile([P, HC * B], dt)
            h2_g = dr.tile([NCORES * P, HC * B], dt)
            nc.sync.dma_start(h2_in[:], h2Tp[:])
            nc.gpsimd.collective_compute(
                "AllGather", mybir.AluOpType.bypass, replica_groups=groups,
                ins=[h2_in.opt()], outs=[h2_g.opt()])
            g2sb = tl.tile([P, NCORES, HC * B], dt)
            nc.gpsimd.dma_start(g2sb[:], h2_g.opt().rearrange("(i p) c -> p i c", p=P))
            h2pre = tl.tile([P, HC * B], dt)
            nc.vector.reduce_sum(h2pre[:], g2sb[:].rearrange("p i c -> p c i"),
                                 axis=mybir.AxisListType.X)
            h2b = tl.tile([P, HC * B], dt)
            for r in range(HC):
                nc.vector.tensor_scalar_add(h2b[:, r * B:(r + 1) * B],
                                            h2pre[:, r * B:(r + 1) * B],
                                            b2T[:, r:r + 1])
            h2a = tl.tile([P, HC * B], dt)
            nc.vector.tensor_scalar_mul(h2a[:], h2b[:], SLOPE)
            h2T = tl.tile([P, HC * B], bt)
            nc.vector.tensor_max(h2T[:], h2b[:], h2a[:])

            # ---- L3: out = h2 @ W3 + b3  [B, F] ----
            out_psum = pp.tile([B, F], dt, tag="tps", bufs=2)
            for r in range(HC):
                nc.tensor.matmul(out_psum[:], lhsT=h2T[:, r * B:(r + 1) * B],
                                 rhs=w3sb[:, r, :], start=(r == 0), stop=False)
            nc.tensor.matmul(out_psum[:], lhsT=ones14[:], rhs=b3row[:],
                             start=False, stop=True)
            out_sb = tl.tile([B, F], dt)
            nc.vector.tensor_copy(out_sb[:], out_psum[:])
            nc.sync.dma_start(out.ap()[:, :], out_sb[:])

            if debug:
                nc.sync.dma_start(dbg["dbg_ssp"].ap()[:, :], ssTp[:])
                nc.sync.dma_start(dbg["dbg_ssum"].ap()[:, :], ssum[:])

    nc.compile()
    return nc


def _get_nc(nsh=N // NCORES, debug=False):
    key = (nsh, debug)
    if key not in _CACHE:
        _CACHE[key] = _build(nsh, debug=debug)
    return _CACHE[key]


def _bf(a):
    import ml_dtypes
    return np.ascontiguousarray(a).astype(ml_dtypes.bfloat16)


def make_in_maps(x, W1, b1, W2, b2, W3, b3, nsh=N // NCORES):
    JSL = D // NCORES
    KC, J2C, HC = D // P, JSL // P, H // P
    x = np.asarray(x, dtype=np.float32)
    W1 = np.asarray(W1, dtype=np.float32)
    b1 = np.asarray(b1, dtype=np.float32)
    W2 = np.asarray(W2, dtype=np.float32)
    b2 = np.asarray(b2, dtype=np.float32)
    W3 = np.asarray(W3, dtype=np.float32)
    b3 = np.asarray(b3, dtype=np.float32)
    w3t = _bf(W3.reshape(HC, P, F).transpose(1, 0, 2))
    b2t = np.ascontiguousarray(b2.reshape(HC, P).T)
    ident = np.eye(B, dtype=np.float32)
    identb = _bf(ident)
    in_maps = []
    for i in range(NCORES):
        xs = np.ascontiguousarray(
            x[:, i * nsh:(i + 1) * nsh, :]).reshape(B * nsh, D)
        w1s = W1[:, i * JSL:(i + 1) * JSL]
        w2s = W2[i * JSL:(i + 1) * JSL, :]
        in_maps.append({
            "x": xs,
            "w1t": _bf(w1s.reshape(KC, P, JSL).transpose(1, 0, 2)),
            "w2t": _bf(w2s.reshape(J2C, P, H).transpose(1, 0, 2)),
            "w3t": w3t,
            "b1r": _bf(b1[i * JSL:(i + 1) * JSL]).reshape(1, JSL),
            "b2tin": b2t, "b3r": _bf(b3).reshape(1, F),
            "ident": ident, "identb": identb,
        })
    return in_maps


def run(x, W1, b1, W2, b2, W3, b3, nsh=N // NCORES, debug=False, trace=False):
    from concourse.bass_utils import run_bass_kernel_spmd
    nc = _get_nc(nsh, debug)
    in_maps = make_in_maps(x, W1, b1, W2, b2, W3, b3, nsh=nsh)
    res = run_bass_kernel_spmd(nc, in_maps, list(range(NCORES)), trace=trace)
    return res


def kernel(x, W1, b1, W2, b2, W3, b3):
    res = run(x, W1, b1, W2, b2, W3, b3)
    return np.asarray(res.results[0]["out"], dtype=np.float32)



# revision 4
# speedup vs baseline: 3.2864x; 3.2864x over previous
"""Trainium2 Bass kernel for nn_CovBlock (B=4, N=8192, D=2048, H=512, F=64).

Data-parallel over 8 NeuronCores: x sharded along N (1024 rows/batch/core).
Per core: one streaming pass over its x shard computing per-column
sum-of-squares of row-centered x, accumulated per batch in PSUM via
TensorE matmuls with a one-hot stationary operand.  The row-mean is folded
into the Square activation (scale=D, bias=-rowsum) so ss is accumulated
scaled by D^2; the eps in cov = ss/(ss+eps) is rescaled to match.

The per-batch partial ss is AllReduce'd in two halves: batches 0-1 fire
mid-pass (latency hidden under the second half of the x stream), batches
2-3 at the end.  The 3-layer MLP is column-sharded (W1) / row-sharded
(W2) across cores with a final small AllReduce for the layer-2 partial
sums.

DMA layout: x streams on the Sync HWDGE ring in 2MB chunks with
contiguous 16KB-per-partition descriptors; small tail DMAs ride the
Scalar (ACT) HWDGE ring; weights + post-collective loads ride GpSimd
(SWDGE), which is allowed to stall on collective completion.
"""

import sys

sys.path.insert(0, "/opt/trn_rl_repo")

import numpy as np

B, N, D, H, F = 4, 8192, 2048, 512, 64
NCORES = 8
P = 128
EPS = 1e-6
SLOPE = 0.01

_CACHE = {}


def _build(nsh, chunk_tiles=2, xbufs=4, sqbufs=3):
    import concourse.bacc as bacc
    import concourse.mybir as mybir
    from concourse import tile

    dt = mybir.dt.float32
    bt = mybir.dt.bfloat16
    AF = mybir.ActivationFunctionType
    ROWS = B * nsh
    NT = ROWS // P            # 32 row tiles per core
    TPB = nsh // P            # 8 tiles per batch
    KC = D // P               # 16 k-chunks of 128
    JSL = D // NCORES         # 256: L1 output column slice per core
    J2C = JSL // P            # 2:  L1-slice k-chunks for L2
    HC = H // P               # 4:  H chunks of 128
    CT = chunk_tiles
    NCH = NT // CT            # 16 chunks
    HALF = NCH // 2           # chunk index where batches 2-3 begin
    EPS2 = EPS * float(D) * float(D)   # ss is accumulated scaled by D^2
    assert NT % CT == 0 and nsh % P == 0 and (CT * P) % nsh == 0 or True
    # each chunk must lie within one batch
    assert nsh % (CT * P) == 0

    nc = bacc.Bacc("TRN2", target_bir_lowering=False, debug=False,
                   num_devices=NCORES)

    x = nc.dram_tensor("x", [ROWS, D], dt, kind="ExternalInput")
    w1t = nc.dram_tensor("w1t", [P, KC, JSL], bt, kind="ExternalInput")
    w2t = nc.dram_tensor("w2t", [P, J2C, H], bt, kind="ExternalInput")
    w3t = nc.dram_tensor("w3t", [P, HC, F], bt, kind="ExternalInput")
    b1r = nc.dram_tensor("b1r", [1, JSL], bt, kind="ExternalInput")
    b2rep = nc.dram_tensor("b2rep", [P, HC * B], dt, kind="ExternalInput")
    b3r = nc.dram_tensor("b3r", [1, F], bt, kind="ExternalInput")
    ident = nc.dram_tensor("ident", [B, B], dt, kind="ExternalInput")
    identb = nc.dram_tensor("identb", [B, B], bt, kind="ExternalInput")
    out = nc.dram_tensor("out", [B, F], dt, kind="ExternalOutput")

    groups = [list(range(NCORES))]

    with tile.TileContext(nc) as tc:
        with (
            tc.tile_pool(name="xp", bufs=xbufs) as xp,
            tc.tile_pool(name="sq", bufs=sqbufs) as sq,
            tc.tile_pool(name="sm", bufs=6) as sm,
            tc.tile_pool(name="wp", bufs=1) as wp,
            tc.tile_pool(name="tl", bufs=1) as tl,
            tc.tile_pool(name="pp", bufs=1, space="PSUM") as pp,
            tc.tile_pool(name="dr", bufs=1, space="DRAM") as dr,
        ):
            # constants: oh2[:, 2j:2j+2] is a [P,2] slice whose column j
            # is all-ones -- selects psum row j for batch parity j.
            oh2 = wp.tile([P, 4], bt)
            nc.any.memset(oh2[:], 0.0)
            nc.any.memset(oh2[:, 0:1], 1.0)
            nc.any.memset(oh2[:, 3:4], 1.0)
            ident4 = wp.tile([B, B], dt)
            nc.gpsimd.dma_start(ident4[:], ident.ap()[:, :])
            ident4b = wp.tile([B, B], bt)
            nc.gpsimd.dma_start(ident4b[:], identb.ap()[:, :])
            ones14 = wp.tile([1, B], bt)
            nc.any.memset(ones14[:], 1.0)

            # weight/bias prefetch on the GpSimd SWDGE ring
            w1sb = wp.tile([P, KC, JSL], bt)
            w2sb = wp.tile([P, J2C, H], bt)
            w3sb = wp.tile([P, HC, F], bt)
            b1row = wp.tile([1, JSL], bt)
            b2T = wp.tile([P, HC * B], dt)
            b3row = wp.tile([1, F], bt)
            nc.gpsimd.dma_start(w1sb[:], w1t.ap()[:, :, :])
            nc.gpsimd.dma_start(w2sb[:], w2t.ap()[:, :, :])
            nc.gpsimd.dma_start(w3sb[:], w3t.ap()[:, :, :])
            nc.gpsimd.dma_start(b1row[:], b1r.ap()[:, :])
            nc.gpsimd.dma_start(b2T[:], b2rep.ap()[:, :])
            nc.gpsimd.dma_start(b3row[:], b3r.ap()[:, :])

            ss_psum = pp.tile([2, D], dt)
            ssT_psum = pp.tile([P, KC * B], dt, tag="sst")

            # DRAM staging for the two ss AllReduces
            ssA_in = dr.tile([2, D], dt)
            ssA_out = dr.tile([2, D], dt)
            ssB_in = dr.tile([2, D], dt)
            ssB_out = dr.tile([2, D], dt)

            ssA_tot = tl.tile([2, D], dt)
            ssB_tot = tl.tile([2, D], dt)

            # ---- main pass over x ----
            for k in range(NCH):
                xch = xp.tile([P, CT, D], dt)
                src = x.ap()[k * CT * P:(k + 1) * CT * P, :]
                nc.sync.dma_start(xch[:], src.rearrange("(p t) d -> p t d", p=P))
                for t in range(CT):
                    g = k * CT + t
                    j = (g // TPB) % 2
                    xt = xch[:, t, :]
                    negsum = sm.tile([P, 1], dt)
                    nc.vector.reduce_sum(negsum[:], xt, axis=mybir.AxisListType.X,
                                         negate=True)
                    xsq = sq.tile([P, D], bt)
                    nc.scalar.activation(xsq[:], xt, AF.Square,
                                         bias=negsum[:], scale=float(D))
                    for q in range(D // 512):
                        nc.tensor.matmul(
                            ss_psum[:, q * 512:(q + 1) * 512],
                            lhsT=oh2[:, 2 * j:2 * j + 2],
                            rhs=xsq[:, q * 512:(q + 1) * 512],
                            start=(g == 0 or g == NT // 2),
                            stop=(g == NT // 2 - 1 or g == NT - 1))

                if k == HALF - 1:
                    # drain batches 0-1, AllReduce hidden under chunks 8..15
                    ssA_sb = tl.tile([2, D], dt)
                    nc.vector.tensor_copy(ssA_sb[:, :D // 2],
                                          ss_psum[0:2, :D // 2])
                    nc.scalar.copy(ssA_sb[:, D // 2:], ss_psum[0:2, D // 2:])
                    nc.scalar.dma_start(ssA_in[:], ssA_sb[:])
                    nc.gpsimd.collective_compute(
                        "AllReduce", mybir.AluOpType.add,
                        replica_groups=groups,
                        ins=[ssA_in.opt()], outs=[ssA_out.opt()])
                    nc.gpsimd.dma_start(ssA_tot[:], ssA_out.opt()[:, :])

            # ---- tail: drain batches 2-3, second AllReduce ----
            ssB_sb = tl.tile([2, D], dt)
            nc.vector.tensor_copy(ssB_sb[:, :D // 2], ss_psum[0:2, :D // 2])
            nc.scalar.copy(ssB_sb[:, D // 2:], ss_psum[0:2, D // 2:])
            nc.scalar.dma_start(ssB_in[:], ssB_sb[:])
            nc.gpsimd.collective_compute(
                "AllReduce", mybir.AluOpType.add, replica_groups=groups,
                ins=[ssB_in.opt()], outs=[ssB_out.opt()])
            nc.gpsimd.dma_start(ssB_tot[:], ssB_out.opt()[:, :])

            # transpose both halves into [P, KC*B] (c-major, batch minor).
            # A's transposes run right after the loop (its data arrived
            # mid-pass); B's run as soon as its AllReduce lands.
            for c in range(KC):
                nc.tensor.transpose(ssT_psum[:, c * B:c * B + 2],
                                    ssA_tot[0:2, c * P:(c + 1) * P],
                                    ident4[0:2, 0:2])
            for c in range(KC):
                nc.tensor.transpose(ssT_psum[:, c * B + 2:c * B + 4],
                                    ssB_tot[0:2, c * P:(c + 1) * P],
                                    ident4[0:2, 0:2])

            # cov = ss/(ss+eps) computed on the transposed layout
            t1 = tl.tile([P, KC * B], dt)
            nc.vector.tensor_scalar_add(t1[:], ssT_psum[:], EPS2)
            t2 = tl.tile([P, KC * B], dt)
            nc.vector.reciprocal(t2[:], t1[:])
            cov = tl.tile([P, KC * B], bt)
            nc.vector.tensor_mul(cov[:], ssT_psum[:], t2[:])

            # ---- L1: h1 = leaky(cov @ W1[:, slice] + b1[slice])  [B, JSL] ----
            h1_psum = pp.tile([B, JSL], dt, tag="tps", bufs=2)
            for c in range(KC):
                nc.tensor.matmul(h1_psum[:], lhsT=cov[:, c * B:(c + 1) * B],
                                 rhs=w1sb[:, c, :], start=(c == 0), stop=False)
            nc.tensor.matmul(h1_psum[:], lhsT=ones14[:], rhs=b1row[:],
                             start=False, stop=True)
            h1a = tl.tile([B, JSL], dt)
            nc.vector.tensor_scalar_mul(h1a[:], h1_psum[:], SLOPE)
            h1_sb = tl.tile([B, JSL], bt)
            nc.vector.tensor_max(h1_sb[:], h1_psum[:], h1a[:])

            h1T_psum = pp.tile([P, J2C * B], bt, tag="tps", bufs=2)
            for cc in range(J2C):
                nc.tensor.transpose(h1T_psum[:, cc * B:(cc + 1) * B],
                                    h1_sb[0:B, cc * P:(cc + 1) * P], ident4b[:])
            h1T = tl.tile([P, J2C * B], bt)
            nc.vector.tensor_copy(h1T[:], h1T_psum[:])

            # ---- L2 partial: h2p = h1 @ W2[slice, :]  [B, H] ----
            h2_psum = pp.tile([B, H], dt, tag="tps", bufs=2)
            for cc in range(J2C):
                nc.tensor.matmul(h2_psum[:], lhsT=h1T[:, cc * B:(cc + 1) * B],
                                 rhs=w2sb[:, cc, :], start=(cc == 0),
                                 stop=(cc == J2C - 1))
            h2p_sb = tl.tile([B, H], dt)
            nc.vector.tensor_copy(h2p_sb[:, :H // 2], h2_psum[:, :H // 2])
            nc.scalar.copy(h2p_sb[:, H // 2:], h2_psum[:, H // 2:])

            h2_in = dr.tile([B, H], dt)
            h2_out = dr.tile([B, H], dt)
            nc.scalar.dma_start(h2_in[:], h2p_sb[:])
            nc.gpsimd.collective_compute(
                "AllReduce", mybir.AluOpType.add, replica_groups=groups,
                ins=[h2_in.opt()], outs=[h2_out.opt()])
            h2tot = tl.tile([B, H], dt)
            nc.gpsimd.dma_start(h2tot[:], h2_out.opt()[:, :])

            h2T_psum = pp.tile([P, HC * B], dt, tag="tps", bufs=2)
            for r in range(HC):
                nc.tensor.transpose(h2T_psum[:, r * B:(r + 1) * B],
                                    h2tot[0:B, r * P:(r + 1) * P], ident4[:])
            h2b = tl.tile([P, HC * B], dt)
            nc.vector.tensor_add(h2b[:], h2T_psum[:], b2T[:])
            h2a = tl.tile([P, HC * B], dt)
            nc.vector.tensor_scalar_mul(h2a[:], h2b[:], SLOPE)
            h2T = tl.tile([P, HC * B], bt)
            nc.vector.tensor_max(h2T[:], h2b[:], h2a[:])

            # ---- L3: out = h2 @ W3 + b3  [B, F] ----
            out_psum = pp.tile([B, F], dt, tag="tps", bufs=2)
            for r in range(HC):
                nc.tensor.matmul(out_psum[:], lhsT=h2T[:, r * B:(r + 1) * B],
                                 rhs=w3sb[:, r, :], start=(r == 0), stop=False)
            nc.tensor.matmul(out_psum[:], lhsT=ones14[:], rhs=b3row[:],
                             start=False, stop=True)
            out_sb = tl.tile([B, F], dt)
            nc.vector.tensor_copy(out_sb[:], out_psum[:])
            nc.sync.dma_start(out.ap()[:, :], out_sb[:])

    nc.compile()
    return nc


def _get_nc(nsh=N // NCORES):
    key = nsh
    if key not in _CACHE:
        _CACHE[key] = _build(nsh)
    return _CACHE[key]


def _bf(a):
    import ml_dtypes
    return np.ascontiguousarray(a).astype(ml_dtypes.bfloat16)


def make_in_maps(x, W1, b1, W2, b2, W3, b3, nsh=N // NCORES):
    JSL = D // NCORES
    KC, J2C, HC = D // P, JSL // P, H // P
    x = np.asarray(x, dtype=np.float32)
    W1 = np.asarray(W1, dtype=np.float32)
    b1 = np.asarray(b1, dtype=np.float32)
    W2 = np.asarray(W2, dtype=np.float32)
    b2 = np.asarray(b2, dtype=np.float32)
    W3 = np.asarray(W3, dtype=np.float32)
    b3 = np.asarray(b3, dtype=np.float32)
    w3t = _bf(W3.reshape(HC, P, F).transpose(1, 0, 2))
    b2rep = np.ascontiguousarray(
        np.repeat(b2.reshape(HC, P).T, B, axis=1)).astype(np.float32)
    ident = np.eye(B, dtype=np.float32)
    identb = _bf(ident)
    in_maps = []
    for i in range(NCORES):
        xs = np.ascontiguousarray(
            x[:, i * nsh:(i + 1) * nsh, :]).reshape(B * nsh, D)
        w1s = W1[:, i * JSL:(i + 1) * JSL]
        w2s = W2[i * JSL:(i + 1) * JSL, :]
        in_maps.append({
            "x": xs,
            "w1t": _bf(w1s.reshape(KC, P, JSL).transpose(1, 0, 2)),
            "w2t": _bf(w2s.reshape(J2C, P, H).transpose(1, 0, 2)),
            "w3t": w3t,
            "b1r": _bf(b1[i * JSL:(i + 1) * JSL]).reshape(1, JSL),
            "b2rep": b2rep, "b3r": _bf(b3).reshape(1, F),
            "ident": ident, "identb": identb,
        })
    return in_maps


def run(x, W1, b1, W2, b2, W3, b3, nsh=N // NCORES, trace=False):
    from concourse.bass_utils import run_bass_kernel_spmd
    nc = _get_nc(nsh)
    in_maps = make_in_maps(x, W1, b1, W2, b2, W3, b3, nsh=nsh)
    res = run_bass_kernel_spmd(nc, in_maps, list(range(NCORES)), trace=trace)
    return res


def kernel(x, W1, b1, W2, b2, W3, b3):
    res = run(x, W1, b1, W2, b2, W3, b3)
    return np.asarray(res.results[0]["out"], dtype=np.float32)
